# revision 22
# baseline (speedup 1.0000x reference)
"""Trainium2 Bass kernel for the CapsuleLayer routing problem.

Strategy (8 NeuronCores, shard the input-capsule dim I; I_loc = 256/core):

  Front (per rep):
    - W is streamed from HBM twice. Pass A computes iter-0's preactivation
      directly: route0 is uniform 1/D, so preact0 = sum_i votes/D is a plain
      matmul of (x/D) against W, PSUM-accumulated over all 32 W tiles with
      all 32 batch rows in one chain. Its AllReduce launches ~12us into the
      rep and hides completely under pass B.
    - Pass B computes votes[b,i,(d,a)] with block-diagonal x as the
      stationary operand (8 capsules x 16 atoms per 128-wide matmul). The
      block-diagonal tile is built on-chip: a persistent zeroed SBUF tile
      plus 8 per-rep strided DMAs writing the 16x16 blocks.
    - votes PSUM tiles are cast to bf16 V [128=(j,bh), G, D, A] by the
      Scalar and Vector engines (alternating).

  Routing iterations (all on-chip, V stays in SBUF):
    - softmax over d: exp on Scalar, sum/reciprocal/normalize on Vector.
    - route-weighted i-reduction: one-hot matmul chains (p_out=16 per
      half), consuming wv = R*V chunks produced by Vector (low g) and
      GpSimd (high g) in parallel.
    - per-iteration cross-core AllReduce of the [32, 512] bf16 partials
      (one merged collective for both halves).
    - squash: Square via Vector self-mult, unit-stride tensor_reduce over
      the innermost a axis (V free layout is (g, d, a)), Sqrt on Scalar.
    - agreement update u = sum_a V*act: Vector does 3/4 of the g-range
      (mult + single tensor_reduce), GpSimd the rest (mult + halving tree).
    - last iteration's partials go out in f32; the host sums the 8 cores,
      adds bias and applies the final squash in float64.

  V/L are double-buffered so rep k+1's front (DMA + PE votes) pipelines
  under rep k's routing tail.
"""

import functools

import numpy as np
import ml_dtypes

import concourse.bass as bass
import concourse.tile as tile
from concourse import bacc, mybir
from concourse import bass_utils

N_CORES = 8
B, I, C, D, A = 32, 2048, 16, 32, 16
I_LOC = I // N_CORES          # 256 capsules per core
G = I_LOC // 8                # 32 groups of 8 capsules
O = A * D                     # 512, free layout is (d, a) with a innermost

F32 = mybir.dt.float32
BF16 = mybir.dt.bfloat16
_nbf16 = ml_dtypes.bfloat16

WVC = 4          # g-groups per wv chunk
WV_GPS = 6       # wv chunks WVC wide handled by DVE; rest by GpSimd
UCH = 8          # g-groups per u chunk


def _build(num_routing: int, reps: int = 1, opts: frozenset = frozenset()):
    n = num_routing
    nc = bacc.Bacc("TRN2", target_bir_lowering=False, debug=False,
                   enable_asserts=True, num_devices=N_CORES)

    w_in = nc.dram_tensor("w", [G, 128, O], BF16, kind="ExternalInput").ap()
    xcj_in = nc.dram_tensor("xcj", [8, 16, G, 2, 16], BF16,
                            kind="ExternalInput").ap()
    xt_in = nc.dram_tensor("xt", [128, G, B], BF16, kind="ExternalInput").ap()
    sh_in = nc.dram_tensor("sh", [128, 16], BF16, kind="ExternalInput").ap()
    repl_in = nc.dram_tensor("repl", [B, 2, 128], BF16,
                             kind="ExternalInput").ap()
    bias2_in = nc.dram_tensor("bias2", [1, O], BF16, kind="ExternalInput").ap()
    ones1_in = nc.dram_tensor("ones1", [1, 128], BF16,
                              kind="ExternalInput").ap()
    outp = nc.dram_tensor("outp", [B, O], BF16, kind="ExternalOutput").ap()

    Exp = mybir.ActivationFunctionType.Exp
    add = mybir.AluOpType.add
    mult = mybir.AluOpType.mult
    AX = mybir.AxisListType.X

    with tile.TileContext(nc) as tc:
        with (
            tc.tile_pool(name="persist", bufs=1) as persist,
            tc.tile_pool(name="vpool", bufs=2) as vpool,
            tc.tile_pool(name="lpool", bufs=1) as lpool,
            tc.tile_pool(name="xtpool", bufs=2) as xtpool,
            tc.tile_pool(name="wpool", bufs=3) as wpool,
            tc.tile_pool(name="pspool", bufs=3, space="PSUM") as pspool,
            tc.tile_pool(name="papool", bufs=1, space="PSUM") as papool,
            tc.tile_pool(name="prpool", bufs=1, space="PSUM") as prpool,
            tc.tile_pool(name="upool", bufs=1) as upool,
            tc.tile_pool(name="wvpool", bufs=2) as wvpool,
            tc.tile_pool(name="stage", bufs=2) as stage,
            tc.tile_pool(name="rpool", bufs=1) as rpool,
            tc.tile_pool(name="small", bufs=2) as small,
            tc.tile_pool(name="outs", bufs=1) as outs,
            tc.tile_pool(name="dram", bufs=3, space="DRAM") as dram,
        ):
            # ---- persistent constants + on-chip block-diagonal x ----
            xd_sb = persist.tile([128, G, 2, 128], BF16, tag="xd", name="xd")
            nc.vector.memset(xd_sb[:], 0.0)
            sh_sb = persist.tile([128, 16], BF16, tag="sh", name="sh_sb")
            nc.sync.dma_start(sh_sb[:], sh_in[:])
            repl_sb = persist.tile([B, 2, 128], BF16, tag="repl", name="repl_sb")
            nc.sync.dma_start(repl_sb[:], repl_in[:])
            bias2_sb = persist.tile([1, O], BF16, tag="bias2", name="bias2_sb")
            nc.sync.dma_start(bias2_sb[:], bias2_in[:])
            ones1_sb = persist.tile([1, 128], BF16, tag="ones1", name="ones1_sb")
            nc.sync.dma_start(ones1_sb[:], ones1_in[:])

            for _rep in range(reps):
                xt_sb = xtpool.tile([128, G, B], BF16, tag="xt", name="xt_sb")
                nc.sync.dma_start(xt_sb[:], xt_in[:])
                for j in range(8):
                    nc.sync.dma_start(
                        xd_sb[bass.ts(j, 16), :, :, bass.ts(j, 16)], xcj_in[j])

                V = [vpool.tile([128, G, D, A], BF16, tag=f"V{h}", name=f"V{h}")
                     for h in range(2)]
                L = [lpool.tile([128, G, D], F32, tag=f"L{h}", name=f"L{h}")
                     for h in range(2)]
                if n > 1:
                    nc.vector.memset(L[0][:], 0.0)
                    nc.gpsimd.memset(L[1][:], 0.0)

                # ---- front: one W pass, g-interleaved so the PE never
                # goes idle (its clock ramps 0.65 -> 1.2 -> 2.4 GHz with
                # sustained use).  Per g: iter-0 partial accumulation
                # (= sum_i votes / D, all 32 b in one PSUM chain) plus the
                # two votes matmuls; PSUM->V casts split ACT/DVE.
                pa0 = papool.tile([B, O], F32, tag="paA", name="pa0")
                for g in range(G):
                    wt = wpool.tile([128, O], BF16, tag="wt", name="wt")
                    eng = nc.sync if g % 2 == 0 else nc.gpsimd
                    eng.dma_start(wt[:], w_in[g])
                    nc.tensor.matmul(pa0[:], lhsT=xt_sb[:, g, :], rhs=wt[:],
                                     start=(g == 0), stop=(g == G - 1))
                    for h in range(2):
                        ps = pspool.tile([128, O], F32, tag="ps", name="ps")
                        nc.tensor.matmul(ps[:], lhsT=xd_sb[:, g, h, :],
                                         rhs=wt[:], start=True, stop=True)
                        dst = V[h][:, g]
                        src = ps[:].rearrange("p (d a) -> p d a", d=D)
                        if h == 0:
                            nc.scalar.copy(dst[:], src)
                        else:
                            nc.vector.tensor_copy(dst[:], src)
                if n == 1:
                    pre0 = outs.tile([B, O], BF16, tag="preA", name="pre0")
                    nc.vector.tensor_copy(pre0[:], pa0[:])
                    nc.sync.dma_start(outp[:], pre0[:])
                    continue
                pre0 = outs.tile([B, O], BF16, tag="preA", name="pre0")
                nc.vector.tensor_copy(pre0[:], pa0[:])
                inb0 = dram.tile([B, O], BF16, tag="arin", name="arin")
                outb0 = dram.tile([B, O], BF16, tag="arout", name="arout",
                                  addr_space="Shared")
                nc.sync.dma_start(inb0[:], pre0[:])
                nc.gpsimd.collective_compute(
                    "AllReduce", add,
                    replica_groups=[list(range(N_CORES))],
                    ins=[inb0[:].opt()], outs=[outb0[:].opt()])

                # ---- routing iterations ----
                ob_prev = outb0
                for t in range(1, n):
                    # tail of iteration t-1: squash + u update,
                    # using the AllReduce result of iteration t-1.
                    ob_sb = outs.tile([B, O], BF16, tag="ob", name="ob_sb")
                    nc.sync.dma_start(ob_sb[:], ob_prev[:])
                    actbs = []
                    for h in range(2):
                        prep = prpool.tile([128, O], F32, tag=f"pr{h}",
                                           name="prep")
                        nc.tensor.matmul(prep[:], lhsT=repl_sb[:, h, :],
                                         rhs=ob_sb[:], start=True, stop=False)
                        nc.tensor.matmul(prep[:], lhsT=ones1_sb[:],
                                         rhs=bias2_sb[:], start=False,
                                         stop=True)
                        t2 = prep[:].rearrange("p (d a) -> p d a", d=D)
                        sq = stage.tile([128, D, A], F32, tag="sq", name="sq")
                        nc.scalar.square(sq[:], t2)
                        n2 = small.tile([128, D], F32, tag="n2", name="n2")
                        nc.vector.tensor_reduce(n2[:], sq[:], axis=AX, op=add)
                        nrm = small.tile([128, D], F32, tag="nrm", name="nrm")
                        nc.scalar.sqrt(nrm[:], n2[:])
                        den = small.tile([128, D], F32, tag="den", name="den")
                        nc.vector.tensor_scalar_add(den[:], n2[:], 1.0)
                        rc2 = small.tile([128, D], F32, tag="rc2", name="rc2")
                        nc.vector.reciprocal(rc2[:], den[:])
                        fac = small.tile([128, D], F32, tag="fac", name="fac")
                        nc.vector.tensor_tensor(fac[:], nrm[:], rc2[:], mult)
                        actb = stage.tile([128, D, A], BF16, tag="actb",
                                          name="actb")
                        nc.vector.tensor_tensor(
                            actb[:], t2,
                            fac[:, :, None].to_broadcast([128, D, A]), mult)
                        actbs.append(actb)
                    # u update: L += sum_a V * actb  (bf16 mult + bf16
                    # halving tree, both at the DVE 2x packed rate; GpSimd
                    # takes 3 of the 8 chunks)
                    for uc in range(G // UCH):
                        for h in range(2):
                            actb = actbs[h]
                            gs = bass.ts(uc, UCH)
                            on_gps = (uc, h) in ((1, 1), (3, 0), (3, 1))
                            eng = nc.gpsimd if on_gps else nc.vector
                            u = upool.tile([128, UCH, D, A], BF16,
                                           tag="ug" if on_gps else "ud",
                                           name="u")
                            eng.tensor_tensor(
                                u[:], V[h][:, gs],
                                actb[:, None, :, :]
                                .to_broadcast([128, UCH, D, A]), mult)
                            half = A // 2
                            while half >= 1:
                                eng.tensor_tensor(
                                    u[:, :, :, 0:half], u[:, :, :, 0:half],
                                    u[:, :, :, half:2 * half], add)
                                half //= 2
                            eng.tensor_tensor(
                                L[h][:, gs], L[h][:, gs], u[:, :, :, 0], add)

                    # iteration t: softmax + route-weighted reduction
                    is_last = (t == n - 1)
                    Rs = []
                    for h in range(2):
                        ex = stage.tile([128, G, D], BF16, tag="ex", name="ex")
                        nc.scalar.activation(ex[:], L[h][:], Exp)
                        sm = small.tile([128, G], F32, tag="sm", name="sm")
                        nc.vector.tensor_reduce(sm[:], ex[:], axis=AX, op=add)
                        rc = small.tile([128, G], F32, tag="rc", name="rc")
                        nc.vector.reciprocal(rc[:], sm[:])
                        R = rpool.tile([128, G, D], BF16, tag=f"R{h}", name="R")
                        nc.vector.tensor_tensor(
                            R[:], ex[:],
                            rc[:, :, None].to_broadcast([128, G, D]), mult)
                        Rs.append(R)
                    pas = []
                    for h in range(2):
                        R = Rs[h]
                        pa = papool.tile([16, O], F32, tag=f"pa{h}", name="pa")
                        for g in range(G):
                            if g % WVC == 0:
                                wc = g // WVC
                                wv = wvpool.tile([128, WVC, D, A], BF16,
                                                 tag="wv", name="wv")
                                eng = nc.vector if wc < WV_GPS else nc.gpsimd
                                eng.tensor_tensor(
                                    wv[:], V[h][:, bass.ts(wc, WVC)],
                                    R[:, bass.ts(wc, WVC), :, None]
                                    .to_broadcast([128, WVC, D, A]), mult)
                            nc.tensor.matmul(pa[:], lhsT=sh_sb[:],
                                             rhs=wv[:, g % WVC],
                                             start=(g == 0), stop=(g == G - 1))
                        pas.append(pa)
                    if is_last:
                        for h in range(2):
                            pre_h = outs.tile([16, O], BF16, tag=f"pre{h}",
                                              name="pre_h")
                            nc.vector.tensor_copy(pre_h[:], pas[h][:])
                            nc.sync.dma_start(outp[bass.ts(h, 16)], pre_h[:])
                        continue
                    inb = dram.tile([B, O], BF16, tag="arin", name="arin")
                    outb = dram.tile([B, O], BF16, tag="arout", name="arout",
                                     addr_space="Shared")
                    for h in range(2):
                        pre_h = outs.tile([16, O], BF16, tag=f"pre{h}",
                                           name="pre_h")
                        nc.vector.tensor_copy(pre_h[:], pas[h][:])
                        nc.sync.dma_start(inb[bass.ts(h, 16)], pre_h[:])
                    nc.gpsimd.collective_compute(
                        "AllReduce", add,
                        replica_groups=[list(range(N_CORES))],
                        ins=[inb[:].opt()], outs=[outb[:].opt()])
                    ob_prev = outb

    nc.compile()
    return nc


KERNEL_OPTS = frozenset()


@functools.lru_cache(maxsize=4)
def _get_compiled(num_routing: int):
    return _build(num_routing, opts=KERNEL_OPTS)


def _host_inputs(x, weights, opts: frozenset = frozenset()):
    """Build the per-core input maps (everything except tiny constants)."""
    x_np = np.ascontiguousarray(x.reshape(B, I, C), dtype=np.float32)
    w2 = np.ascontiguousarray(weights.reshape(I, C, O), dtype=np.float32)

    in_maps = []
    for r in range(N_CORES):
        sl = slice(r * I_LOC, (r + 1) * I_LOC)
        w_r = w2[sl].reshape(G, 128, O).astype(_nbf16)
        # arr[h, bh, g, j, c] = x[h*16+bh, r*I_LOC + g*8 + j, c]
        arr = x_np[:, sl, :].reshape(2, 16, G, 8, C)
        arr_bf = arr.astype(_nbf16)
        # xcj[j, c, g, h, bh]
        xcj = np.ascontiguousarray(arr_bf.transpose(3, 4, 2, 0, 1))
        # xt[(j c), g, b] = x[b, g*8+j, c] / D   (D=32 exact in bf16)
        xt = np.ascontiguousarray(
            (arr.astype(np.float32) / D).transpose(3, 4, 2, 0, 1)
            .reshape(128, G, B)).astype(_nbf16)
        in_maps.append({"w": np.ascontiguousarray(w_r), "xcj": xcj, "xt": xt})
    return in_maps


def _host_constants(bias):
    sh = np.zeros((128, 16), np.float32)
    for j in range(8):
        for bh in range(16):
            sh[j * 16 + bh, bh] = 1.0
    repl = np.zeros((B, 2, 128), np.float32)
    for h in range(2):
        for j in range(8):
            for bh in range(16):
                repl[h * 16 + bh, h, j * 16 + bh] = 1.0
    bias2 = np.ascontiguousarray(bias.reshape(D * A), dtype=np.float32)
    return {"sh": sh.astype(_nbf16), "repl": repl.astype(_nbf16),
            "bias2": bias2[None, :].astype(_nbf16),
            "ones1": np.ones((1, 128), _nbf16)}


def _squash_host(t):
    # t: [B, D, A] float64; squash over a
    n2 = (t ** 2).sum(axis=2, keepdims=True)
    nrm = np.sqrt(n2)
    return t * (nrm / (1.0 + n2))


def kernel(x, weights, bias, num_routing):
    n = int(num_routing)
    x = np.asarray(x, dtype=np.float32)
    weights = np.asarray(weights, dtype=np.float32)
    bias_np = np.asarray(bias, dtype=np.float32)

    nc = _get_compiled(n)
    in_maps = _host_inputs(x, weights, opts=KERNEL_OPTS)
    consts = _host_constants(bias_np)
    for m in in_maps:
        m.update(consts)

    # the axon tunnel occasionally returns a transient
    # NRT_EXEC_UNIT_UNRECOVERABLE; one retry has recovered every observed case
    import time as _time
    try:
        res = bass_utils.run_bass_kernel_spmd(
            nc, in_maps, core_ids=list(range(N_CORES)))
    except Exception:
        _time.sleep(10)
        res = bass_utils.run_bass_kernel_spmd(
            nc, in_maps, core_ids=list(range(N_CORES)))

    partials = np.stack([res.results[r]["outp"] for r in range(N_CORES)], axis=0)
    pre = partials.astype(np.float64).sum(axis=0)            # [B, O] in (d, a)
    pre = pre.reshape(B, D, A)
    pre = pre + bias_np.reshape(D, A)[None].astype(np.float64)
    act = _squash_host(pre).astype(np.float32)
    return act.reshape(B, D, A, 1, 1)


if __name__ == "__main__":
    import sys
    sys.path.insert(0, "/root/problem")
    from reference import setup_inputs, reference

    inputs = {k: np.asarray(v) if not isinstance(v, int) else v
              for k, v in setup_inputs().items()}
    ref = np.asarray(reference(**inputs))
    out = kernel(**inputs)
    d = np.abs(out - ref)
    print("absmax", d.max(), "ref absmax", np.abs(ref).max(),
          "scale-rel", d.max() / np.abs(ref).max(),
          "rel_l2", np.linalg.norm(d) / np.linalg.norm(ref))
